# revision 17
# baseline (speedup 1.0000x reference)
"""Trainium2 Bass kernel for a pre-LN transformer block (B=2, T=2048, C=512,
H=16 heads, HS=32, DF=2048), distributed over 8 NeuronCores.

Sharding strategy (v2):
  Cores are split into 2 groups of 4 by batch (cores 0-3 -> batch 0,
  cores 4-7 -> batch 1). Each core:
   - Phase 0: replicates LN1 over ALL 2048 tokens of its batch (no
     collective needed; LN gain/bias are folded into the QKV weights
     host-side) and PE-transposes to hT [C, 2048] bf16.
   - Phase 1: computes q^T,k^T [128, 2048] and v for its 4 heads.
   - Phase 1b: causal attention in transposed-score space; the per-head
     softmax denominator Z comes from a ones-column in the [v|1]
     stationary; 1/Z is folded in at the source via a small PE broadcast
     matmul, so the AllToAll payload is normalized o in fp8 (256KB total).
   - AllToAll (8-core mesh): head-sharded -> token-sharded, where each
     core owns token slab [256c, 256c+256) of BOTH batches so the
     collective is fully dense.
   - Phase 2: Wo + residual + LN2 + FFN for its 512 tokens.
  All LN gains/biases and bo are folded host-side (diag(g)@W, be@W + b).
"""
import numpy as np

import bass_rust
import concourse.bass as bass
import concourse.mybir as mybir
import concourse.tile as tile
from concourse.bass_utils import run_bass_kernel_spmd

B, T, C, H, HS = 2, 2048, 512, 16, 32
DF = 4 * C
EPS = 1e-3
NCORES = 8
GROUP = 4           # cores per batch group
HPC = H // GROUP    # 4 heads per core
D2 = HPC * HS       # 128 = packed head dim per core
TB = 512            # token chunk for QKV/attention loops
QT = 256            # token slab per core for phase 2 (per batch)
P = 128
NCT = C // P        # 4 c-tiles
NFT = DF // P       # 16 f-tiles
NTT = T // P        # 16 token tiles per batch
NST = T // P        # 16 s-tiles per batch
F32 = mybir.dt.float32
F32R = mybir.dt.float32r
BF16 = mybir.dt.bfloat16
FP8 = mybir.dt.float8e4
I16 = mybir.dt.int16
AF = mybir.ActivationFunctionType
ALU = mybir.AluOpType
WS = 16.0  # fp8 weight pre-scale (avoids subnormal truncation); undone at outputs
# exp(s) ~ bitcast_bf16(int16(s * SCH_A + SCH_B)), max rel err ~3.3%
SCH_A = 184.6649652337873
SCH_B = 16251.0
# uniform score offset: e' = exp(s - SOFF) keeps fp8 e in range; softmax-invariant
SOFF = 0.0

_ev_counter = [0]


def _split_excess_waits(nc, max_waits=1):
    """This walrus build rejects >1 semaphore wait per real instruction; Tile's
    kernel-tail drain (and occasionally other aggregation points) can exceed
    that. Hoist extra waits onto EventSemaphore instructions inserted
    immediately before, on the same engine."""
    n_split = 0
    for bb in nc.main_func.blocks:
        il = bb.instructions
        i = 0
        while i < len(il):
            inst = il[i]
            si = inst.sync_info
            if si is None:
                i += 1
                continue
            waits = list(si.on_wait)
            if len(waits) <= max_waits:
                i += 1
                continue
            keep, extra = waits[:max_waits], waits[max_waits:]
            evs = []
            for w in extra:
                _ev_counter[0] += 1
                ev = mybir.InstEventSemaphore(
                    name=f"EV-WSPLIT-{_ev_counter[0]}",
                    engine=inst.engine,
                    sync_info=bass_rust.SyncInfo(on_wait=[w], on_update=[]),
                )
                nc.register_instruction(ev)
                evs.append(ev)
            inst.sync_info = bass_rust.SyncInfo(
                on_wait=keep, on_update=list(si.on_update)
            )
            for k, ev in enumerate(evs):
                il.insert(i + k, ev)
            i += len(evs) + 1
            n_split += 1
    return n_split


def _build_nc(repeat=1, skip=()):
    nc = bass.Bass(num_devices=NCORES)

    # ---- per-core external inputs ----
    xfull = nc.declare_dram_parameter("xfull", [T, C], F32, isOutput=False)
    xres = nc.declare_dram_parameter("xres", [2 * QT, C], F32, isOutput=False)
    wq = nc.declare_dram_parameter("wq", [C, D2], F32, isOutput=False)
    wk = nc.declare_dram_parameter("wk", [C, D2], F32, isOutput=False)
    wv = nc.declare_dram_parameter("wv", [C, D2], F32, isOutput=False)
    bq = nc.declare_dram_parameter("bq", [D2], F32, isOutput=False)
    bk = nc.declare_dram_parameter("bk", [D2], F32, isOutput=False)
    bv = nc.declare_dram_parameter("bv", [D2], F32, isOutput=False)
    # fp8 DoubleRow layouts: w[k, u, j, n] = W[128*(2u+j)+k, n]
    wo8 = nc.declare_dram_parameter("wo8", [P, 2, 2, C], FP8, isOutput=False)
    w18 = nc.declare_dram_parameter("w18", [P, 2, 2, DF], FP8, isOutput=False)
    b1r = nc.declare_dram_parameter("b1r", [DF // P, P], F32, isOutput=False)
    w2b = nc.declare_dram_parameter("w2b", [DF, C], BF16, isOutput=False)
    b2 = nc.declare_dram_parameter("b2", [C], F32, isOutput=False)
    out = nc.declare_dram_parameter("out", [2 * QT, C], F32, isOutput=True)

    ident_dram = nc.inline_tensor(np.eye(P, dtype=np.float32), name="ident_c")
    # E4[r, p] = 1 iff p // 32 == r  (broadcast 1/Z row r to its 32 partitions)
    e4 = np.zeros((HPC, P), dtype=np.float32)
    for r in range(HPC):
        e4[r, 32 * r : 32 * r + 32] = 1.0
    e4_dram = nc.inline_tensor(e4, name="e4_c")

    with tile.TileContext(nc) as tc:
        import contextlib

        with contextlib.ExitStack() as ctx:
            const = ctx.enter_context(tc.tile_pool(name="const", bufs=1))
            persist = ctx.enter_context(tc.tile_pool(name="persist", bufs=2))
            dram = ctx.enter_context(tc.tile_pool(name="dram", bufs=2, space="DRAM"))

            # ---- constants ----
            identb = const.tile([P, P], BF16, name="identb")
            ident_st = const.tile([P, P], F32, name="ident_st")
            nc.sync.dma_start(out=ident_st, in_=ident_dram[:, :])
            nc.vector.tensor_copy(identb, ident_st)
            eps_t = const.tile([P, 1], F32, name="eps_t")
            nc.vector.memset(eps_t, EPS)
            # LN2 runs on WS-scaled x2; var scales by WS^2, so eps must too
            eps2_t = const.tile([P, 1], F32, name="eps2_t")
            nc.vector.memset(eps2_t, EPS * WS * WS)
            e4_sb = const.tile([HPC, P], F32, name="e4_sb")
            nc.sync.dma_start(out=e4_sb, in_=e4_dram[:, :])
            ones_row = const.tile([1, P], BF16, name="ones_row")
            nc.vector.memset(ones_row, 1.0)
            b2_sb = const.tile([1, C], BF16, name="b2_sb")
            b2_st = const.tile([1, C], F32, name="b2_st")
            nc.sync.dma_start(out=b2_st, in_=b2[:].unsqueeze(0))
            nc.vector.tensor_scalar(
                out=b2_sb, in0=b2_st, scalar1=WS, scalar2=None, op0=ALU.mult
            )
            soff_t = const.tile([P, 1], F32, name="soff_t")
            nc.vector.memset(soff_t, -SOFF)
            ones512 = const.tile([1, TB], BF16, name="ones512")
            nc.vector.memset(ones512, 1.0)
            bq_row = const.tile([1, D2], BF16, name="bq_row")
            bk_row = const.tile([1, D2], BF16, name="bk_row")
            bqk_st = const.tile([1, 2 * D2], F32, name="bqk_st")
            nc.sync.dma_start(out=bqk_st[:, 0:D2], in_=bq[:].unsqueeze(0))
            nc.sync.dma_start(out=bqk_st[:, D2 : 2 * D2], in_=bk[:].unsqueeze(0))
            nc.vector.tensor_copy(bq_row, bqk_st[:, 0:D2])
            nc.vector.tensor_copy(bk_row, bqk_st[:, D2 : 2 * D2])
            bvb = const.tile([P, D2], F32, name="bvb")
            nc.sync.dma_start(out=bvb, in_=bv[:].partition_broadcast(P))
            b1cols = const.tile([P, NFT], F32, name="b1cols")
            for f in range(NFT):
                nc.sync.dma_start(out=b1cols[:, f : f + 1], in_=b1r[f, :].unsqueeze(1))
            # FFN runs WS-scaled end-to-end: gT = relu(ps_g + WS*b1)
            nc.vector.tensor_scalar(
                out=b1cols, in0=b1cols, scalar1=WS, scalar2=None, op0=ALU.mult
            )

            # ---- weights resident in SBUF ----
            wq_sb = const.tile([P, NCT, D2], BF16, name="wq_sb")
            wk_sb = const.tile([P, NCT, D2], BF16, name="wk_sb")
            wv_sb = const.tile([P, NCT, D2], BF16, name="wv_sb")
            wqkv_st = const.tile([P, 3 * D2], F32, name="wqkv_st")
            for j in range(NCT):
                nc.sync.dma_start(out=wqkv_st[:, 0:D2], in_=wq[j * P : (j + 1) * P, :])
                nc.sync.dma_start(
                    out=wqkv_st[:, D2 : 2 * D2], in_=wk[j * P : (j + 1) * P, :]
                )
                nc.sync.dma_start(
                    out=wqkv_st[:, 2 * D2 : 3 * D2], in_=wv[j * P : (j + 1) * P, :]
                )
                nc.vector.tensor_copy(wq_sb[:, j, :], wqkv_st[:, 0:D2])
                nc.vector.tensor_copy(wk_sb[:, j, :], wqkv_st[:, D2 : 2 * D2])
                nc.vector.tensor_copy(wv_sb[:, j, :], wqkv_st[:, 2 * D2 : 3 * D2])
            wo_sb = const.tile([P, 2, 2, C], FP8, name="wo_sb")
            nc.sync.dma_start(out=wo_sb[:, :, :, :], in_=wo8[:, :, :, :])
            w1_sb = const.tile([P, 2, 2, DF], FP8, name="w1_sb")
            nc.sync.dma_start(out=w1_sb[:, :, :, :], in_=w18[:, :, :, :])
            w2_sb = const.tile([P, NFT, C], BF16, name="w2_sb")
            for f in range(NFT):
                nc.sync.dma_start(out=w2_sb[:, f, :], in_=w2b[f * P : (f + 1) * P, :])

            p1big = ctx.enter_context(tc.tile_pool(name="p1big", bufs=1))
            exp_rr = [0]

            def emit_front():
                """LN1 + QKV + q/k remap for one iteration; pipelined so it
                can overlap the previous iteration's AllToAll + phase 2."""
                # ======== Phase 0: replicated LN1 over my batch + transpose ====
                x_sb = persist.tile([P, 2 * QT // P, C], F32, name="x_sb")  # residual
                nc.sync.dma_start(out=x_sb[:, 0, :], in_=xres[0:P, :])
                nc.sync.dma_start(out=x_sb[:, 1, :], in_=xres[P : 2 * P, :])
                nc.sync.dma_start(out=x_sb[:, 2, :], in_=xres[2 * P : 3 * P, :])
                nc.sync.dma_start(out=x_sb[:, 3, :], in_=xres[3 * P : 4 * P, :])

                hT = p1big.tile([P, NCT, T], BF16, name="hT")
                with (
                    tc.tile_pool(name="ph0", bufs=6) as ph0,
                    tc.tile_pool(name="ph0ps", bufs=8, space="PSUM") as ph0ps,
                ):
                    for i in range(NTT):
                        x_t = ph0.tile([P, C], F32, name="x_t0")
                        nc.sync.dma_start(out=x_t, in_=xfull[i * P : (i + 1) * P, :])
                        stats = ph0.tile([P, 6], F32, name="stats0")
                        nc.vector.bn_stats(out=stats, in_=x_t)
                        mv = ph0.tile([P, 2], F32, name="mv0")
                        nc.vector.bn_aggr(out=mv, in_=stats)
                        rstd = ph0.tile([P, 1], F32, name="rstd0")
                        nc.scalar.activation(
                            out=rstd, in_=mv[:, 1:2], func=AF.Sqrt, bias=eps_t
                        )
                        nc.vector.reciprocal(out=rstd, in_=rstd)
                        nmr = ph0.tile([P, 1], F32, name="nmr0")
                        nc.vector.tensor_scalar(
                            out=nmr,
                            in0=mv[:, 0:1],
                            scalar1=rstd,
                            scalar2=-1.0,
                            op0=ALU.mult,
                            op1=ALU.mult,
                        )
                        h_t = ph0.tile([P, C], BF16, name="h_t0")
                        nc.gpsimd.tensor_scalar(
                            out=h_t,
                            in0=x_t,
                            scalar1=rstd,
                            scalar2=nmr,
                            op0=ALU.mult,
                            op1=ALU.add,
                        )
                        tr_ps = ph0ps.tile([P, NCT, P], BF16, name="tr_ps0")
                        for j in range(NCT):
                            nc.tensor.transpose(
                                tr_ps[:, j, :], h_t[:, j * P : (j + 1) * P], identb[:]
                            )
                        nc.scalar.activation(
                            out=hT[:, :, i * P : (i + 1) * P],
                            in_=tr_ps[:, :, :],
                            func=AF.Copy,
                        )

                # ======== Phase 1: QKV for my 4 heads over my batch ============
                # q8full/k8full are head-major: partition 32*hh+hs, so head hh
                # lives on partition strip [32hh, 32hh+32) — directly usable as
                # row-tiled score-matmul operands (no remap needed).
                q8full = p1big.tile([P, T], FP8, name="q8full")
                k8full = p1big.tile([P, T], FP8, name="k8full")
                # v layout: [128 s-part, s-tile 16, head 4, 33]; col 32 = ones
                v_sb = p1big.tile([P, NST, HPC, HS + 1], BF16, name="v_sb")
                nc.vector.memset(v_sb[:, :, :, HS : HS + 1], 1.0)

                with (
                    tc.tile_pool(name="ph1", bufs=2) as ph1,
                    tc.tile_pool(name="ph1ps", bufs=2, space="PSUM") as ph1ps,
                ):
                    for n in range(T // TB):  # 4 chunks of 512 tokens
                        ps_q = ph1ps.tile([D2, TB], F32, name="ps_q")
                        for j in range(NCT):
                            nc.tensor.matmul(
                                ps_q[:],
                                wq_sb[:, j, :],
                                hT[:, j, n * TB : (n + 1) * TB],
                                start=(j == 0),
                                stop=False,
                            )
                        nc.tensor.matmul(
                            ps_q[:], bq_row[0:1, :], ones512[0:1, :],
                            start=False, stop=True,
                        )
                        nc.scalar.activation(
                            out=q8full[:, n * TB : (n + 1) * TB],
                            in_=ps_q[:],
                            func=AF.Copy,
                        )
                        ps_k = ph1ps.tile([D2, TB], F32, name="ps_k")
                        for j in range(NCT):
                            nc.tensor.matmul(
                                ps_k[:],
                                wk_sb[:, j, :],
                                hT[:, j, n * TB : (n + 1) * TB],
                                start=(j == 0),
                                stop=False,
                            )
                        nc.tensor.matmul(
                            ps_k[:], bk_row[0:1, :], ones512[0:1, :],
                            start=False, stop=True,
                        )
                        nc.scalar.activation(
                            out=k8full[:, n * TB : (n + 1) * TB],
                            in_=ps_k[:],
                            func=AF.Copy,
                        )
                        for m in range(TB // P):
                            g = n * (TB // P) + m  # global s-tile index
                            ps_v = ph1ps.tile([P, D2], F32, name="ps_v")
                            for j in range(NCT):
                                nc.tensor.matmul(
                                    ps_v[:],
                                    hT[:, j, g * P : (g + 1) * P],
                                    wv_sb[:, j, :],
                                    start=(j == 0),
                                    stop=(j == NCT - 1),
                                )
                            nc.vector.tensor_tensor(
                                out=v_sb[:, g, :, 0:HS],
                                in0=ps_v[:],
                                in1=bvb,
                                op=ALU.add,
                            )

                return dict(
                    x_sb=x_sb, hT=hT, q8=q8full, k8=k8full, v_sb=v_sb
                )

            def emit_attn_pack(st):
                v_sb = st["v_sb"]
                q8 = st["q8"]
                k8 = st["k8"]
                # o+Z staging, packed 2 heads per bank-column: head 2*hp+u at
                # partitions [64u, 64u+33); row 32+64u = that head's Z.
                st33 = p1big.tile([P, 2, 4, TB], BF16, name="st33")
                st8 = p1big.tile([P, 2, 4, TB], FP8, name="st8")
                zd = dram.tile([HPC * T], BF16, name="zd")
                zr = dram.tile([HPC * T], BF16, name="zr")
                rmat32 = p1big.tile([P, 2, T], BF16, name="rmat32")
                # per-repeat DRAM comm buffers
                a2a_in = dram.tile([NCORES, HS, HPC, QT], FP8, name="a2a_in")
                a2a_out = dram.tile([NCORES, HS, HPC, QT], FP8, name="a2a_out")
                # [128, 16] views of the Z scratch for batched reciprocal:
                # chunk ci -> partition 32h+q holds Z[head h, tok 16q..16q+16)
                zdv = zd[:].rearrange("(c h q f) -> c (h q) f", c=4, h=HPC, q=32, f=16)
                zrv = zr[:].rearrange("(c h q f) -> c (h q) f", c=4, h=HPC, q=32, f=16)

                # ======== Phase 1b: causal attention, transposed-score space ===
                if "attn" in skip:
                    nc.gpsimd.memset(st8[:, :, :, :], 1.0)
                with (
                    tc.tile_pool(name="att", bufs=5) as att,
                    tc.tile_pool(name="attps", bufs=2, space="PSUM") as attps,
                    tc.tile_pool(name="attpso", bufs=1, space="PSUM") as attpso,
                ):
                    SCH_B2 = SCH_B - SOFF * SCH_A

                    def emit_exp(dst, src):
                        exp_rr[0] += 1
                        if exp_rr[0] % 2 == 0:
                            # Schraudolph exp on DVE: bf16 via int16
                            nc.vector.tensor_scalar(
                                out=dst.bitcast(I16),
                                in0=src,
                                scalar1=SCH_A,
                                scalar2=SCH_B2,
                                op0=ALU.mult,
                                op1=ALU.add,
                            )
                        else:
                            nc.scalar.activation(out=dst, in_=src, func=AF.Exp)

                    def emit_av(ps_o, j, ns, e2, o0):
                        for hp in range(2):
                            for u in range(2):
                                hh = 2 * hp + u
                                nc.tensor.matmul(
                                    ps_o[64 * u : 64 * u + HS + 1, hh, o0:TB],
                                    v_sb[:, j, hh, :],
                                    e2[:, hp, u, o0:TB],
                                    start=(j == 0),
                                    stop=(j == ns - 1),
                                )

                    for ci in range(4 if "attn" not in skip else 0):
                        t0 = ci * TB
                        ns = 4 * ci + 4  # s-tiles for this chunk
                        ps_o = attpso.tile([P, 4, TB], F32, name="ps_o")
                        pend = None  # previous j's (j, e2, o0) awaiting av
                        for j in range(ns):
                            off = j - 4 * ci
                            o0 = max(off, 0) * P
                            pss = []
                            for hp in range(2):
                                ps_s = attps.tile(
                                    [P, 2, TB], F32, name="ps_s"
                                )
                                for u in range(2):
                                    hh = 2 * hp + u
                                    # row-tiled: head hh on PE strip 32*hh
                                    nc.tensor.matmul(
                                        ps_s[:, u, o0:TB],
                                        k8[32 * hh : 32 * hh + 32, j * P : (j + 1) * P],
                                        q8[32 * hh : 32 * hh + 32, t0 + o0 : t0 + TB],
                                        start=True,
                                        stop=True,
                                        tile_position=(32 * hh, 0),
                                    )
                                pss.append(ps_s)
                            # av of j-1 goes AFTER j's score matmuls in the PE
                            # queue so next scores aren't stuck behind it
                            if pend is not None:
                                emit_av(ps_o, *pend)
                            e2 = att.tile([P, 2, 2, TB], BF16, name="e2")
                            for hp in range(2):
                                emit_exp(
                                    e2[:, hp, :, o0:TB], pss[hp][:, :, o0:TB]
                                )
                            if off >= 0:
                                for hp in range(2):
                                    # mask both heads' [128,128] diag blocks
                                    nc.gpsimd.affine_select(
                                        out=e2[:, hp, :, o0 : o0 + P],
                                        in_=e2[:, hp, :, o0 : o0 + P],
                                        compare_op=ALU.is_ge,
                                        fill=0.0,
                                        base=0,
                                        pattern=[[0, 2], [1, P]],
                                        channel_multiplier=-1,
                                    )
                            pend = (j, ns, e2, o0)
                        emit_av(ps_o, *pend)
                        # drain chunk: copy o+Z rows, batched 1/Z, fold, stage
                        for hh2 in range(HPC):
                            hp2, u2 = hh2 // 2, hh2 % 2
                            dst = st33[64 * u2 : 64 * u2 + HS + 1, hp2, ci, :]
                            so = ps_o[64 * u2 : 64 * u2 + HS + 1, hh2, :]
                            if hh2 % 2 == 0:
                                nc.scalar.activation(out=dst, in_=so, func=AF.Copy)
                            else:
                                nc.vector.tensor_copy(dst, so)
                        for hh in range(HPC):
                            hp, u = hh // 2, hh % 2
                            zsl = slice((ci * HPC + hh) * TB, (ci * HPC + hh + 1) * TB)
                            nc.sync.dma_start(
                                out=zd[zsl].unsqueeze(0),
                                in_=st33[32 + 64 * u : 33 + 64 * u, hp, ci, :],
                            )
                        zsb = att.tile([P, 16], BF16, name="zsb")
                        rzsb = att.tile([P, 16], BF16, name="rzsb")
                        nc.sync.dma_start(out=zsb, in_=zdv[ci])
                        with nc.allow_low_precision(reason="1/Z in bf16"):
                            nc.vector.reciprocal(out=rzsb, in_=zsb)
                        nc.sync.dma_start(out=zrv[ci], in_=rzsb)
                        csl = slice(ci * TB, (ci + 1) * TB)
                        for hh in range(HPC):
                            hp, u = hh // 2, hh % 2
                            zsl = slice((ci * HPC + hh) * TB, (ci * HPC + hh + 1) * TB)
                            nc.sync.dma_start(
                                out=rmat32[64 * u : 64 * u + HS, hp, csl],
                                in_=zr[zsl].partition_broadcast(HS),
                            )
                        for u in range(2):
                            eng = nc.vector if u else nc.gpsimd
                            eng.tensor_tensor(
                                out=st8[64 * u : 64 * u + HS, :, ci, :],
                                in0=st33[64 * u : 64 * u + HS, :, ci, :],
                                in1=rmat32[64 * u : 64 * u + HS, :, csl],
                                op=ALU.mult,
                            )
                        # pack this chunk's A2A payload now (dst cores 2ci and
                        # 2ci+1 own these tokens). Payload head-slot order is
                        # [0, 2, 1, 3]; the host Wo row permutation matches.
                        for k in (2 * ci, 2 * ci + 1):
                            qsl = slice((k % 2) * QT, (k % 2 + 1) * QT)
                            nc.sync.dma_start(
                                out=a2a_in[k, :, 0:2, :],
                                in_=st8[0:HS, :, ci, qsl],
                            )
                            nc.sync.dma_start(
                                out=a2a_in[k, :, 2:4, :],
                                in_=st8[64 : 64 + HS, :, ci, qsl],
                            )

                if "a2a" in skip:
                    nc.sync.dma_start(
                        out=a2a_out[:, :, :, :], in_=a2a_in[:, :, :, :]
                    )
                else:
                    nc.gpsimd.collective_compute(
                        "AllToAll",
                        ALU.bypass,
                        replica_groups=[list(range(NCORES))],
                        ins=[a2a_in[:, :, :, :]],
                        outs=[a2a_out[:, :, :, :]],
                    )

                return a2a_out

            def emit_ph2(st, a2a_out):
                x_sb = st["x_sb"]
                # ======== Phase 2: Wo + residual + LN2 + FFN ===================
                with (
                    tc.tile_pool(name="ph2", bufs=4) as ph2,
                    tc.tile_pool(name="ph2w", bufs=1) as ph2w,
                ):
                    x2_sb = ph2w.tile([P, 2 * QT // P, C], F32, name="x2_sb")
                    h2T8 = ph2w.tile([P, NCT, 2 * QT], FP8, name="h2T8")
                    with tc.tile_pool(name="ph2psA", bufs=4, space="PSUM") as ph2psA:
                        for s2 in range(2):  # batch slab
                            # unpack: oT[32*jsrc+hs, hh, col] =
                            #   a2a_out[4*s2+jsrc, 32*hh+hs, col]
                            oT8 = ph2.tile([P, HPC, QT], FP8, name="oT8")
                            for jsrc in range(GROUP):
                                nc.sync.dma_start(
                                    out=oT8[32 * jsrc : 32 * jsrc + 32, :, :],
                                    in_=a2a_out[s2 * GROUP + jsrc, :, :, :],
                                )
                            for m in range(QT // P):
                                mi = 2 * s2 + m  # tile index within my 512 tokens
                                ps_a = ph2psA.tile([P, C], F32, name="ps_a")
                                for u in range(2):
                                    nc.tensor.matmul(
                                        ps_a[:],
                                        oT8[:, 2 * u : 2 * u + 2, m * P : (m + 1) * P],
                                        wo_sb[:, u, :, :],
                                        perf_mode=mybir.MatmulPerfMode.DoubleRow,
                                        start=(u == 0),
                                        stop=(u == 1),
                                    )
                                nc.vector.tensor_tensor(
                                    out=x2_sb[:, mi, :],
                                    in0=ps_a[:],
                                    in1=x_sb[:, mi, :],
                                    op=ALU.add,
                                )
                                stats2 = ph2.tile([P, 6], F32, name="stats2")
                                nc.vector.bn_stats(out=stats2, in_=x2_sb[:, mi, :])
                                mv2 = ph2.tile([P, 2], F32, name="mv2")
                                nc.vector.bn_aggr(out=mv2, in_=stats2)
                                rstd2 = ph2.tile([P, 1], F32, name="rstd2")
                                nc.scalar.activation(
                                    out=rstd2, in_=mv2[:, 1:2], func=AF.Sqrt, bias=eps2_t
                                )
                                nc.vector.reciprocal(out=rstd2, in_=rstd2)
                                nmr2 = ph2.tile([P, 1], F32, name="nmr2")
                                nc.vector.tensor_scalar(
                                    out=nmr2,
                                    in0=mv2[:, 0:1],
                                    scalar1=rstd2,
                                    scalar2=-1.0,
                                    op0=ALU.mult,
                                    op1=ALU.mult,
                                )
                                h2_t = ph2.tile([P, C], BF16, name="h2_t")
                                nc.gpsimd.tensor_scalar(
                                    out=h2_t,
                                    in0=x2_sb[:, mi, :],
                                    scalar1=rstd2,
                                    scalar2=nmr2,
                                    op0=ALU.mult,
                                    op1=ALU.add,
                                )
                                tr_ps = ph2psA.tile([P, NCT, P], BF16, name="tr_ps2")
                                for j in range(NCT):
                                    nc.tensor.transpose(
                                        tr_ps[:, j, :],
                                        h2_t[:, j * P : (j + 1) * P],
                                        identb[:],
                                    )
                                nc.scalar.activation(
                                    out=h2T8[:, :, mi * P : (mi + 1) * P],
                                    in_=tr_ps[:, :, :],
                                    func=AF.Copy,
                                )

                    # FFN1 fp8 DoubleRow; FFN2 bf16 (error budget), WS-scaled
                    gT = ph2w.tile([P, NFT, 2 * QT], BF16, name="gT")
                    if "ffn" in skip:
                        for m in range(2 * QT // P):
                            y_t = ph2.tile([P, C], F32, name="y_t")
                            nc.vector.tensor_copy(y_t, x2_sb[:, m, :])
                            nc.sync.dma_start(out=out[m * P : (m + 1) * P, :], in_=y_t)
                        return
                    with (
                        tc.tile_pool(name="ph2psB", bufs=4, space="PSUM") as ph2psB,
                        tc.tile_pool(name="ph2psY", bufs=1, space="PSUM") as ph2psY,
                    ):
                        ps_y = ph2psY.tile([P, 2 * QT // P, C], F32, name="ps_y")
                        for f in range(NFT):
                            ps_g = ph2psB.tile([P, 2 * QT], F32, name="ps_g")
                            for u in range(2):
                                nc.tensor.matmul(
                                    ps_g[:],
                                    w1_sb[:, u, :, f * P : (f + 1) * P],
                                    h2T8[:, 2 * u : 2 * u + 2, :],
                                    perf_mode=mybir.MatmulPerfMode.DoubleRow,
                                    start=(u == 0),
                                    stop=(u == 1),
                                )
                            nc.scalar.activation(
                                out=gT[:, f, :],
                                in_=ps_g[:],
                                func=AF.Relu,
                                bias=b1cols[:, f : f + 1],
                            )
                            for m in range(2 * QT // P):
                                nc.tensor.matmul(
                                    ps_y[:, m, :],
                                    gT[:, f, m * P : (m + 1) * P],
                                    w2_sb[:, f, :],
                                    start=(f == 0),
                                    stop=False,
                                )
                        for m in range(2 * QT // P):
                            # rank-1 b2 add closes the accumulation group
                            nc.tensor.matmul(
                                ps_y[:, m, :],
                                ones_row[0:1, 0:P],
                                b2_sb[0:1, :],
                                start=False,
                                stop=True,
                            )
                            y_t = ph2.tile([P, C], F32, name="y_t")
                            nc.vector.tensor_tensor(
                                out=y_t, in0=ps_y[:, m, :], in1=x2_sb[:, m, :], op=ALU.add
                            )
                            nc.vector.tensor_scalar(
                                out=y_t, in0=y_t, scalar1=1.0 / WS, scalar2=None,
                                op0=ALU.mult,
                            )
                            nc.sync.dma_start(out=out[m * P : (m + 1) * P, :], in_=y_t)


            st = emit_front()
            for _rep in range(repeat):
                a2a_out_r = emit_attn_pack(st)
                nxt = emit_front() if _rep + 1 < repeat else None
                emit_ph2(st, a2a_out_r)
                st = nxt

    _split_excess_waits(nc)
    return nc


_NC_CACHE = None


def _get_nc():
    global _NC_CACHE
    if _NC_CACHE is None:
        _NC_CACHE = _build_nc()
    return _NC_CACHE


def _make_in_maps(inputs):
    f = lambda a: np.ascontiguousarray(np.asarray(a, dtype=np.float32))
    x = f(inputs["x"])  # [B, T, C]
    Wq, Wk, Wv = f(inputs["Wq"]), f(inputs["Wk"]), f(inputs["Wv"])
    bq, bk, bv = f(inputs["bq"]), f(inputs["bk"]), f(inputs["bv"])
    Wo, bo = f(inputs["Wo"]), f(inputs["bo"])
    W1, b1 = f(inputs["W1"]), f(inputs["b1"])
    W2, b2 = f(inputs["W2"]), f(inputs["b2"])
    g1, be1 = f(inputs["g1"]), f(inputs["be1"])
    g2, be2 = f(inputs["g2"]), f(inputs["be2"])

    # LN1 fold: h = z*g1 + be1  =>  h@W = z@(g1 d W) + be1@W
    Wq_f = g1[:, None] * Wq  # [H, C, HS] broadcast over H? shape [H,C,HS]
    Wk_f = g1[:, None] * Wk
    Wv_f = g1[:, None] * Wv
    # per-head folded biases
    bq_f = np.einsum("c,hcd->hd", be1, Wq) + bq
    bk_f = np.einsum("c,hcd->hd", be1, Wk) + bk
    bv_f = np.einsum("c,hcd->hd", be1, Wv) + bv
    # LN2 fold into FFN1
    W1_f = g2[:, None] * W1
    b1_f = be2 @ W1 + b1
    # wo row permutation: payload head-slot order is [0, 2, 1, 3] (partition-
    # group major from the packed st8 layout); slot hidx of src core jsrc
    # carries true head head_at[hidx], i.e.
    # c' = 128*hidx + 32*jsrc + hs <- c = 32*(4*jsrc + head_at[hidx]) + hs
    head_at = [0, 2, 1, 3]
    perm = np.empty(C, dtype=np.int64)
    for hidx in range(HPC):
        for jsrc in range(GROUP):
            for hs in range(HS):
                perm[128 * hidx + 32 * jsrc + hs] = (
                    32 * (4 * jsrc + head_at[hidx]) + hs
                )
    Wo_p = np.ascontiguousarray(Wo[perm])

    import ml_dtypes

    def dr_layout(W):  # [R, N] -> [128, R//256, 2, N] fp8: w[k,u,j,n]=W[128*(2u+j)+k,n]
        R, N = W.shape
        w = W.reshape(R // 256, 2, P, N).transpose(2, 0, 1, 3)
        return np.ascontiguousarray(w.astype(ml_dtypes.float8_e4m3))

    shared = {
        "wo8": dr_layout(Wo_p * WS),
        "w18": dr_layout(W1_f * WS),
        "b1r": np.ascontiguousarray(b1_f).reshape(DF // P, P),
        "w2b": np.ascontiguousarray(W2.astype(ml_dtypes.bfloat16)),
        "b2": b2,
    }
    in_maps = []
    for c in range(NCORES):
        g = c // GROUP
        h0 = HPC * (c % GROUP)
        hsl = slice(h0, h0 + HPC)
        xres = (
            np.concatenate(
                [x[0, QT * c : QT * (c + 1)], x[1, QT * c : QT * (c + 1)]], axis=0
            )
            + bo[None, :]
        ) * WS
        in_maps.append(
            {
                "xfull": x[g],
                "xres": np.ascontiguousarray(xres),
                "wq": np.ascontiguousarray(
                    Wq_f[hsl].transpose(1, 0, 2).reshape(C, D2)
                ),
                "wk": np.ascontiguousarray(
                    Wk_f[hsl].transpose(1, 0, 2).reshape(C, D2)
                ),
                "wv": np.ascontiguousarray(
                    Wv_f[hsl].transpose(1, 0, 2).reshape(C, D2)
                ),
                "bq": np.ascontiguousarray(bq_f[hsl].reshape(-1)),
                "bk": np.ascontiguousarray(bk_f[hsl].reshape(-1)),
                "bv": np.ascontiguousarray(bv_f[hsl].reshape(-1)),
                **shared,
            }
        )
    return in_maps


def kernel(**inputs) -> np.ndarray:
    nc = _get_nc()
    in_maps = _make_in_maps(inputs)
    res = run_bass_kernel_spmd(nc, in_maps, list(range(NCORES)))
    out = np.empty((B, T, C), dtype=np.float32)
    for c in range(NCORES):
        r = res.results[c]["out"]
        out[0, QT * c : QT * (c + 1)] = r[0:QT]
        out[1, QT * c : QT * (c + 1)] = r[QT : 2 * QT]
    return out



# revision 19
# speedup vs baseline: 1.1747x; 1.1747x over previous
"""Trainium2 Bass kernel for a pre-LN transformer block (B=2, T=2048, C=512,
H=16 heads, HS=32, DF=2048), distributed over 8 NeuronCores.

Sharding strategy (v2):
  Cores are split into 2 groups of 4 by batch (cores 0-3 -> batch 0,
  cores 4-7 -> batch 1). Each core:
   - Phase 0: replicates LN1 over ALL 2048 tokens of its batch (no
     collective needed; LN gain/bias are folded into the QKV weights
     host-side) and PE-transposes to hT [C, 2048] bf16.
   - Phase 1: computes q^T,k^T [128, 2048] and v for its 4 heads.
   - Phase 1b: causal attention in transposed-score space; the per-head
     softmax denominator Z comes from a ones-column in the [v|1]
     stationary; 1/Z is folded in at the source via a small PE broadcast
     matmul, so the AllToAll payload is normalized o in fp8 (256KB total).
   - AllToAll (8-core mesh): head-sharded -> token-sharded, where each
     core owns token slab [256c, 256c+256) of BOTH batches so the
     collective is fully dense.
   - Phase 2: Wo + residual + LN2 + FFN for its 512 tokens.
  All LN gains/biases and bo are folded host-side (diag(g)@W, be@W + b).
"""
import numpy as np

import bass_rust
import concourse.bass as bass
import concourse.mybir as mybir
import concourse.tile as tile
from concourse.bass_utils import run_bass_kernel_spmd

B, T, C, H, HS = 2, 2048, 512, 16, 32
DF = 4 * C
EPS = 1e-3
NCORES = 8
GROUP = 4           # cores per batch group
HPC = H // GROUP    # 4 heads per core
D2 = HPC * HS       # 128 = packed head dim per core
TB = 512            # token chunk for QKV/attention loops
QT = 256            # token slab per core for phase 2 (per batch)
P = 128
NCT = C // P        # 4 c-tiles
NFT = DF // P       # 16 f-tiles
NTT = T // P        # 16 token tiles per batch
NST = T // P        # 16 s-tiles per batch
F32 = mybir.dt.float32
F32R = mybir.dt.float32r
BF16 = mybir.dt.bfloat16
FP8 = mybir.dt.float8e4
I16 = mybir.dt.int16
AF = mybir.ActivationFunctionType
ALU = mybir.AluOpType
WS = 16.0  # fp8 weight pre-scale (avoids subnormal truncation); undone at outputs
# exp(s) ~ bitcast_bf16(int16(s * SCH_A + SCH_B)), max rel err ~3.3%
SCH_A = 184.6649652337873
SCH_B = 16251.0
# uniform score offset: e' = exp(s - SOFF) keeps fp8 e in range; softmax-invariant
SOFF = 0.0

_ev_counter = [0]


def _split_excess_waits(nc, max_waits=1):
    """This walrus build rejects >1 semaphore wait per real instruction; Tile's
    kernel-tail drain (and occasionally other aggregation points) can exceed
    that. Hoist extra waits onto EventSemaphore instructions inserted
    immediately before, on the same engine."""
    n_split = 0
    for bb in nc.main_func.blocks:
        il = bb.instructions
        i = 0
        while i < len(il):
            inst = il[i]
            si = inst.sync_info
            if si is None:
                i += 1
                continue
            waits = list(si.on_wait)
            if len(waits) <= max_waits:
                i += 1
                continue
            keep, extra = waits[:max_waits], waits[max_waits:]
            evs = []
            for w in extra:
                _ev_counter[0] += 1
                ev = mybir.InstEventSemaphore(
                    name=f"EV-WSPLIT-{_ev_counter[0]}",
                    engine=inst.engine,
                    sync_info=bass_rust.SyncInfo(on_wait=[w], on_update=[]),
                )
                nc.register_instruction(ev)
                evs.append(ev)
            inst.sync_info = bass_rust.SyncInfo(
                on_wait=keep, on_update=list(si.on_update)
            )
            for k, ev in enumerate(evs):
                il.insert(i + k, ev)
            i += len(evs) + 1
            n_split += 1
    return n_split


def _build_nc(repeat=1, skip=()):
    nc = bass.Bass(num_devices=NCORES)

    # ---- per-core external inputs ----
    xfull = nc.declare_dram_parameter("xfull", [T, C], F32, isOutput=False)
    xres = nc.declare_dram_parameter("xres", [2 * QT, C], F32, isOutput=False)
    wq = nc.declare_dram_parameter("wq", [C, D2], F32, isOutput=False)
    wk = nc.declare_dram_parameter("wk", [C, D2], F32, isOutput=False)
    wv = nc.declare_dram_parameter("wv", [C, D2], F32, isOutput=False)
    bq = nc.declare_dram_parameter("bq", [D2], F32, isOutput=False)
    bk = nc.declare_dram_parameter("bk", [D2], F32, isOutput=False)
    bv = nc.declare_dram_parameter("bv", [D2], F32, isOutput=False)
    # fp8 DoubleRow layouts: w[k, u, j, n] = W[128*(2u+j)+k, n]
    wo8 = nc.declare_dram_parameter("wo8", [P, 2, 2, C], FP8, isOutput=False)
    w18 = nc.declare_dram_parameter("w18", [P, 2, 2, DF], FP8, isOutput=False)
    b1r = nc.declare_dram_parameter("b1r", [DF // P, P], F32, isOutput=False)
    w2b = nc.declare_dram_parameter("w2b", [DF, C], BF16, isOutput=False)
    b2 = nc.declare_dram_parameter("b2", [C], F32, isOutput=False)
    out = nc.declare_dram_parameter("out", [2 * QT, C], F32, isOutput=True)

    ident_dram = nc.inline_tensor(np.eye(P, dtype=np.float32), name="ident_c")
    # E4[r, p] = 1 iff p // 32 == r  (broadcast 1/Z row r to its 32 partitions)
    e4 = np.zeros((HPC, P), dtype=np.float32)
    for r in range(HPC):
        e4[r, 32 * r : 32 * r + 32] = 1.0
    e4_dram = nc.inline_tensor(e4, name="e4_c")

    with tile.TileContext(nc) as tc:
        import contextlib

        with contextlib.ExitStack() as ctx:
            const = ctx.enter_context(tc.tile_pool(name="const", bufs=1))
            persist = ctx.enter_context(tc.tile_pool(name="persist", bufs=2))
            dram = ctx.enter_context(tc.tile_pool(name="dram", bufs=2, space="DRAM"))

            # ---- constants ----
            identb = const.tile([P, P], BF16, name="identb")
            ident_st = const.tile([P, P], F32, name="ident_st")
            nc.sync.dma_start(out=ident_st, in_=ident_dram[:, :])
            nc.vector.tensor_copy(identb, ident_st)
            eps_t = const.tile([P, 1], F32, name="eps_t")
            nc.vector.memset(eps_t, EPS)
            # LN2 runs on WS-scaled x2; var scales by WS^2, so eps must too
            eps2_t = const.tile([P, 1], F32, name="eps2_t")
            nc.vector.memset(eps2_t, EPS * WS * WS)
            e4_sb = const.tile([HPC, P], F32, name="e4_sb")
            nc.sync.dma_start(out=e4_sb, in_=e4_dram[:, :])
            ones_row = const.tile([1, P], BF16, name="ones_row")
            nc.vector.memset(ones_row, 1.0)
            b2_sb = const.tile([1, C], BF16, name="b2_sb")
            b2_st = const.tile([1, C], F32, name="b2_st")
            nc.sync.dma_start(out=b2_st, in_=b2[:].unsqueeze(0))
            nc.vector.tensor_scalar(
                out=b2_sb, in0=b2_st, scalar1=WS, scalar2=None, op0=ALU.mult
            )
            soff_t = const.tile([P, 1], F32, name="soff_t")
            nc.vector.memset(soff_t, -SOFF)
            ones512 = const.tile([1, TB], BF16, name="ones512")
            nc.vector.memset(ones512, 1.0)
            bq_row = const.tile([1, D2], BF16, name="bq_row")
            bk_row = const.tile([1, D2], BF16, name="bk_row")
            bqk_st = const.tile([1, 2 * D2], F32, name="bqk_st")
            nc.sync.dma_start(out=bqk_st[:, 0:D2], in_=bq[:].unsqueeze(0))
            nc.sync.dma_start(out=bqk_st[:, D2 : 2 * D2], in_=bk[:].unsqueeze(0))
            nc.vector.tensor_copy(bq_row, bqk_st[:, 0:D2])
            nc.vector.tensor_copy(bk_row, bqk_st[:, D2 : 2 * D2])
            bvb = const.tile([P, D2], F32, name="bvb")
            nc.sync.dma_start(out=bvb, in_=bv[:].partition_broadcast(P))
            b1cols = const.tile([P, NFT], F32, name="b1cols")
            for f in range(NFT):
                nc.sync.dma_start(out=b1cols[:, f : f + 1], in_=b1r[f, :].unsqueeze(1))
            # FFN runs WS-scaled end-to-end: gT = relu(ps_g + WS*b1)
            nc.vector.tensor_scalar(
                out=b1cols, in0=b1cols, scalar1=WS, scalar2=None, op0=ALU.mult
            )

            # ---- weights resident in SBUF ----
            wq_sb = const.tile([P, NCT, D2], BF16, name="wq_sb")
            wk_sb = const.tile([P, NCT, D2], BF16, name="wk_sb")
            wv_sb = const.tile([P, NCT, D2], BF16, name="wv_sb")
            wqkv_st = const.tile([P, 3 * D2], F32, name="wqkv_st")
            for j in range(NCT):
                nc.sync.dma_start(out=wqkv_st[:, 0:D2], in_=wq[j * P : (j + 1) * P, :])
                nc.sync.dma_start(
                    out=wqkv_st[:, D2 : 2 * D2], in_=wk[j * P : (j + 1) * P, :]
                )
                nc.sync.dma_start(
                    out=wqkv_st[:, 2 * D2 : 3 * D2], in_=wv[j * P : (j + 1) * P, :]
                )
                nc.vector.tensor_copy(wq_sb[:, j, :], wqkv_st[:, 0:D2])
                nc.vector.tensor_copy(wk_sb[:, j, :], wqkv_st[:, D2 : 2 * D2])
                nc.vector.tensor_copy(wv_sb[:, j, :], wqkv_st[:, 2 * D2 : 3 * D2])
            wo_sb = const.tile([P, 2, 2, C], FP8, name="wo_sb")
            nc.sync.dma_start(out=wo_sb[:, :, :, :], in_=wo8[:, :, :, :])
            w1_sb = const.tile([P, 2, 2, DF], FP8, name="w1_sb")
            nc.sync.dma_start(out=w1_sb[:, :, :, :], in_=w18[:, :, :, :])
            w2_sb = const.tile([P, NFT, C], BF16, name="w2_sb")
            for f in range(NFT):
                nc.sync.dma_start(out=w2_sb[:, f, :], in_=w2b[f * P : (f + 1) * P, :])

            p1big = ctx.enter_context(tc.tile_pool(name="p1big", bufs=1))
            exp_rr = [0]

            def emit_front():
                """LN1 + QKV + q/k remap for one iteration; pipelined so it
                can overlap the previous iteration's AllToAll + phase 2."""
                # ======== Phase 0: replicated LN1 over my batch + transpose ====
                x_sb = persist.tile([P, 2 * QT // P, C], F32, name="x_sb")  # residual
                nc.sync.dma_start(out=x_sb[:, 0, :], in_=xres[0:P, :])
                nc.sync.dma_start(out=x_sb[:, 1, :], in_=xres[P : 2 * P, :])
                nc.sync.dma_start(out=x_sb[:, 2, :], in_=xres[2 * P : 3 * P, :])
                nc.sync.dma_start(out=x_sb[:, 3, :], in_=xres[3 * P : 4 * P, :])

                hT = p1big.tile([P, NCT, T], BF16, name="hT")
                with (
                    tc.tile_pool(name="ph0", bufs=6) as ph0,
                    tc.tile_pool(name="ph0ps", bufs=8, space="PSUM") as ph0ps,
                ):
                    for i in range(NTT):
                        x_t = ph0.tile([P, C], F32, name="x_t0")
                        nc.sync.dma_start(out=x_t, in_=xfull[i * P : (i + 1) * P, :])
                        stats = ph0.tile([P, 6], F32, name="stats0")
                        nc.vector.bn_stats(out=stats, in_=x_t)
                        mv = ph0.tile([P, 2], F32, name="mv0")
                        nc.vector.bn_aggr(out=mv, in_=stats)
                        rstd = ph0.tile([P, 1], F32, name="rstd0")
                        nc.scalar.activation(
                            out=rstd, in_=mv[:, 1:2], func=AF.Sqrt, bias=eps_t
                        )
                        nc.vector.reciprocal(out=rstd, in_=rstd)
                        nmr = ph0.tile([P, 1], F32, name="nmr0")
                        nc.vector.tensor_scalar(
                            out=nmr,
                            in0=mv[:, 0:1],
                            scalar1=rstd,
                            scalar2=-1.0,
                            op0=ALU.mult,
                            op1=ALU.mult,
                        )
                        h_t = ph0.tile([P, C], BF16, name="h_t0")
                        nc.gpsimd.tensor_scalar(
                            out=h_t,
                            in0=x_t,
                            scalar1=rstd,
                            scalar2=nmr,
                            op0=ALU.mult,
                            op1=ALU.add,
                        )
                        tr_ps = ph0ps.tile([P, NCT, P], BF16, name="tr_ps0")
                        for j in range(NCT):
                            nc.tensor.transpose(
                                tr_ps[:, j, :], h_t[:, j * P : (j + 1) * P], identb[:]
                            )
                        nc.scalar.activation(
                            out=hT[:, :, i * P : (i + 1) * P],
                            in_=tr_ps[:, :, :],
                            func=AF.Copy,
                        )

                # ======== Phase 1: QKV for my 4 heads over my batch ============
                # q8full/k8full are head-major: partition 32*hh+hs, so head hh
                # lives on partition strip [32hh, 32hh+32) — directly usable as
                # row-tiled score-matmul operands (no remap needed).
                q8full = p1big.tile([P, T], FP8, name="q8full")
                k8full = p1big.tile([P, T], FP8, name="k8full")
                # v layout: [128 s-part, s-tile 16, head 4, 33]; col 32 = ones
                v_sb = p1big.tile([P, NST, HPC, HS + 1], BF16, name="v_sb")
                nc.vector.memset(v_sb[:, :, :, HS : HS + 1], 1.0)

                with (
                    tc.tile_pool(name="ph1", bufs=2) as ph1,
                    tc.tile_pool(name="ph1ps", bufs=2, space="PSUM") as ph1ps,
                ):
                    for n in range(T // TB):  # 4 chunks of 512 tokens
                        ps_q = ph1ps.tile([D2, TB], F32, name="ps_q")
                        for j in range(NCT):
                            nc.tensor.matmul(
                                ps_q[:],
                                wq_sb[:, j, :],
                                hT[:, j, n * TB : (n + 1) * TB],
                                start=(j == 0),
                                stop=False,
                            )
                        nc.tensor.matmul(
                            ps_q[:], bq_row[0:1, :], ones512[0:1, :],
                            start=False, stop=True,
                        )
                        nc.scalar.activation(
                            out=q8full[:, n * TB : (n + 1) * TB],
                            in_=ps_q[:],
                            func=AF.Copy,
                        )
                        ps_k = ph1ps.tile([D2, TB], F32, name="ps_k")
                        for j in range(NCT):
                            nc.tensor.matmul(
                                ps_k[:],
                                wk_sb[:, j, :],
                                hT[:, j, n * TB : (n + 1) * TB],
                                start=(j == 0),
                                stop=False,
                            )
                        nc.tensor.matmul(
                            ps_k[:], bk_row[0:1, :], ones512[0:1, :],
                            start=False, stop=True,
                        )
                        nc.scalar.activation(
                            out=k8full[:, n * TB : (n + 1) * TB],
                            in_=ps_k[:],
                            func=AF.Copy,
                        )
                        for m in range(TB // P):
                            g = n * (TB // P) + m  # global s-tile index
                            ps_v = ph1ps.tile([P, D2], F32, name="ps_v")
                            for j in range(NCT):
                                nc.tensor.matmul(
                                    ps_v[:],
                                    hT[:, j, g * P : (g + 1) * P],
                                    wv_sb[:, j, :],
                                    start=(j == 0),
                                    stop=(j == NCT - 1),
                                )
                            nc.vector.tensor_tensor(
                                out=v_sb[:, g, :, 0:HS],
                                in0=ps_v[:],
                                in1=bvb,
                                op=ALU.add,
                            )

                return dict(
                    x_sb=x_sb, hT=hT, q8=q8full, k8=k8full, v_sb=v_sb
                )

            def emit_attn_pack(st):
                v_sb = st["v_sb"]
                q8 = st["q8"]
                k8 = st["k8"]
                # o+Z staging, packed 2 heads per bank-column: head 2*hp+u at
                # partitions [64u, 64u+33); row 32+64u = that head's Z.
                st33 = p1big.tile([P, 2, 4, TB], BF16, name="st33")
                st8 = p1big.tile([P, 2, 4, TB], FP8, name="st8")
                zd = dram.tile([HPC * T], BF16, name="zd")
                zr = dram.tile([HPC * T], BF16, name="zr")
                rmat32 = p1big.tile([P, 2, T], BF16, name="rmat32")
                # per-repeat DRAM comm buffers
                a2a_in = dram.tile([NCORES, HS, HPC, QT], FP8, name="a2a_in")
                a2a_out = dram.tile([NCORES, HS, HPC, QT], FP8, name="a2a_out")
                # [128, 16] views of the Z scratch for batched reciprocal:
                # chunk ci -> partition 32h+q holds Z[head h, tok 16q..16q+16)
                zdv = zd[:].rearrange("(c h q f) -> c (h q) f", c=4, h=HPC, q=32, f=16)
                zrv = zr[:].rearrange("(c h q f) -> c (h q) f", c=4, h=HPC, q=32, f=16)

                # ======== Phase 1b: causal attention, transposed-score space ===
                if "attn" in skip:
                    nc.gpsimd.memset(st8[:, :, :, :], 1.0)
                with (
                    tc.tile_pool(name="att", bufs=5) as att,
                    tc.tile_pool(name="attps", bufs=4, space="PSUM") as attps,
                    tc.tile_pool(name="attpso", bufs=1, space="PSUM") as attpso,
                ):
                    SCH_B2 = SCH_B - SOFF * SCH_A

                    def emit_exp(dst, src, on_dve):
                        if on_dve:
                            # Schraudolph exp on DVE: bf16 via int16
                            nc.vector.tensor_scalar(
                                out=dst.bitcast(I16),
                                in0=src,
                                scalar1=SCH_A,
                                scalar2=SCH_B2,
                                op0=ALU.mult,
                                op1=ALU.add,
                            )
                        else:
                            nc.scalar.activation(out=dst, in_=src, func=AF.Exp)

                    def emit_av(ps_o, j, ns, e2, o0):
                        for hp in range(2):
                            for u in range(2):
                                hh = 2 * hp + u
                                nc.tensor.matmul(
                                    ps_o[64 * u : 64 * u + HS + 1, hh, o0:TB],
                                    v_sb[:, j, hh, :],
                                    e2[:, hp, u, o0:TB],
                                    start=(j == 0),
                                    stop=(j == ns - 1),
                                )

                    for ci in range(4 if "attn" not in skip else 0):
                        t0 = ci * TB
                        ns = 4 * ci + 4  # s-tiles for this chunk
                        ps_o = attpso.tile([P, 4, TB], F32, name="ps_o")
                        pend = None  # previous j's (j, e2, o0) awaiting av
                        for j in range(ns):
                            off = j - 4 * ci
                            o0 = max(off, 0) * P
                            pss = []
                            for hh in range(HPC):
                                ps_s = attps.tile([P, TB], F32, name="ps_s")
                                # row-tiled: head hh on PE strip 32*hh
                                nc.tensor.matmul(
                                    ps_s[:, o0:TB],
                                    k8[32 * hh : 32 * hh + 32, j * P : (j + 1) * P],
                                    q8[32 * hh : 32 * hh + 32, t0 + o0 : t0 + TB],
                                    start=True,
                                    stop=True,
                                    tile_position=(32 * hh, 0),
                                )
                                pss.append(ps_s)
                            # av of j-1 goes AFTER j's score matmuls in the PE
                            # queue so next scores aren't stuck behind it
                            if pend is not None:
                                emit_av(ps_o, *pend)
                            e2 = att.tile([P, 2, 2, TB], BF16, name="e2")
                            for hh in range(HPC):
                                hp, u = hh // 2, hh % 2
                                emit_exp(
                                    e2[:, hp, u, o0:TB],
                                    pss[hh][:, o0:TB],
                                    on_dve=(u == 1),
                                )
                            if off >= 0:
                                for hp in range(2):
                                    # mask both heads' [128,128] diag blocks
                                    nc.gpsimd.affine_select(
                                        out=e2[:, hp, :, o0 : o0 + P],
                                        in_=e2[:, hp, :, o0 : o0 + P],
                                        compare_op=ALU.is_ge,
                                        fill=0.0,
                                        base=0,
                                        pattern=[[0, 2], [1, P]],
                                        channel_multiplier=-1,
                                    )
                            pend = (j, ns, e2, o0)
                        emit_av(ps_o, *pend)
                        # drain chunk: copy o+Z rows, batched 1/Z, fold, stage
                        for hh2 in range(HPC):
                            hp2, u2 = hh2 // 2, hh2 % 2
                            dst = st33[64 * u2 : 64 * u2 + HS + 1, hp2, ci, :]
                            so = ps_o[64 * u2 : 64 * u2 + HS + 1, hh2, :]
                            if hh2 % 2 == 0:
                                nc.scalar.activation(out=dst, in_=so, func=AF.Copy)
                            else:
                                nc.vector.tensor_copy(dst, so)
                        for hh in range(HPC):
                            hp, u = hh // 2, hh % 2
                            zsl = slice((ci * HPC + hh) * TB, (ci * HPC + hh + 1) * TB)
                            nc.sync.dma_start(
                                out=zd[zsl].unsqueeze(0),
                                in_=st33[32 + 64 * u : 33 + 64 * u, hp, ci, :],
                            )
                        zsb = att.tile([P, 16], BF16, name="zsb")
                        rzsb = att.tile([P, 16], BF16, name="rzsb")
                        nc.sync.dma_start(out=zsb, in_=zdv[ci])
                        with nc.allow_low_precision(reason="1/Z in bf16"):
                            nc.vector.reciprocal(out=rzsb, in_=zsb)
                        nc.sync.dma_start(out=zrv[ci], in_=rzsb)
                        csl = slice(ci * TB, (ci + 1) * TB)
                        for hh in range(HPC):
                            hp, u = hh // 2, hh % 2
                            zsl = slice((ci * HPC + hh) * TB, (ci * HPC + hh + 1) * TB)
                            nc.sync.dma_start(
                                out=rmat32[64 * u : 64 * u + HS, hp, csl],
                                in_=zr[zsl].partition_broadcast(HS),
                            )
                        for u in range(2):
                            eng = nc.gpsimd
                            eng.tensor_tensor(
                                out=st8[64 * u : 64 * u + HS, :, ci, :],
                                in0=st33[64 * u : 64 * u + HS, :, ci, :],
                                in1=rmat32[64 * u : 64 * u + HS, :, csl],
                                op=ALU.mult,
                            )
                        # pack this chunk's A2A payload now (dst cores 2ci and
                        # 2ci+1 own these tokens). Payload head-slot order is
                        # [0, 2, 1, 3]; the host Wo row permutation matches.
                        for k in (2 * ci, 2 * ci + 1):
                            qsl = slice((k % 2) * QT, (k % 2 + 1) * QT)
                            nc.sync.dma_start(
                                out=a2a_in[k, :, 0:2, :],
                                in_=st8[0:HS, :, ci, qsl],
                            )
                            nc.sync.dma_start(
                                out=a2a_in[k, :, 2:4, :],
                                in_=st8[64 : 64 + HS, :, ci, qsl],
                            )

                if "a2a" in skip:
                    nc.sync.dma_start(
                        out=a2a_out[:, :, :, :], in_=a2a_in[:, :, :, :]
                    )
                else:
                    nc.gpsimd.collective_compute(
                        "AllToAll",
                        ALU.bypass,
                        replica_groups=[list(range(NCORES))],
                        ins=[a2a_in[:, :, :, :]],
                        outs=[a2a_out[:, :, :, :]],
                    )

                return a2a_out

            def emit_ph2(st, a2a_out):
                x_sb = st["x_sb"]
                # ======== Phase 2: Wo + residual + LN2 + FFN ===================
                with (
                    tc.tile_pool(name="ph2", bufs=4) as ph2,
                    tc.tile_pool(name="ph2w", bufs=1) as ph2w,
                ):
                    x2_sb = ph2w.tile([P, 2 * QT // P, C], F32, name="x2_sb")
                    h2T8 = ph2w.tile([P, NCT, 2 * QT], FP8, name="h2T8")
                    with tc.tile_pool(name="ph2psA", bufs=4, space="PSUM") as ph2psA:
                        for s2 in range(2):  # batch slab
                            # unpack: oT[32*jsrc+hs, hh, col] =
                            #   a2a_out[4*s2+jsrc, 32*hh+hs, col]
                            oT8 = ph2.tile([P, HPC, QT], FP8, name="oT8")
                            for jsrc in range(GROUP):
                                nc.sync.dma_start(
                                    out=oT8[32 * jsrc : 32 * jsrc + 32, :, :],
                                    in_=a2a_out[s2 * GROUP + jsrc, :, :, :],
                                )
                            for m in range(QT // P):
                                mi = 2 * s2 + m  # tile index within my 512 tokens
                                ps_a = ph2psA.tile([P, C], F32, name="ps_a")
                                for u in range(2):
                                    nc.tensor.matmul(
                                        ps_a[:],
                                        oT8[:, 2 * u : 2 * u + 2, m * P : (m + 1) * P],
                                        wo_sb[:, u, :, :],
                                        perf_mode=mybir.MatmulPerfMode.DoubleRow,
                                        start=(u == 0),
                                        stop=(u == 1),
                                    )
                                nc.vector.tensor_tensor(
                                    out=x2_sb[:, mi, :],
                                    in0=ps_a[:],
                                    in1=x_sb[:, mi, :],
                                    op=ALU.add,
                                )
                                stats2 = ph2.tile([P, 6], F32, name="stats2")
                                nc.vector.bn_stats(out=stats2, in_=x2_sb[:, mi, :])
                                mv2 = ph2.tile([P, 2], F32, name="mv2")
                                nc.vector.bn_aggr(out=mv2, in_=stats2)
                                rstd2 = ph2.tile([P, 1], F32, name="rstd2")
                                nc.scalar.activation(
                                    out=rstd2, in_=mv2[:, 1:2], func=AF.Sqrt, bias=eps2_t
                                )
                                nc.vector.reciprocal(out=rstd2, in_=rstd2)
                                nmr2 = ph2.tile([P, 1], F32, name="nmr2")
                                nc.vector.tensor_scalar(
                                    out=nmr2,
                                    in0=mv2[:, 0:1],
                                    scalar1=rstd2,
                                    scalar2=-1.0,
                                    op0=ALU.mult,
                                    op1=ALU.mult,
                                )
                                h2_t = ph2.tile([P, C], BF16, name="h2_t")
                                nc.gpsimd.tensor_scalar(
                                    out=h2_t,
                                    in0=x2_sb[:, mi, :],
                                    scalar1=rstd2,
                                    scalar2=nmr2,
                                    op0=ALU.mult,
                                    op1=ALU.add,
                                )
                                tr_ps = ph2psA.tile([P, NCT, P], BF16, name="tr_ps2")
                                for j in range(NCT):
                                    nc.tensor.transpose(
                                        tr_ps[:, j, :],
                                        h2_t[:, j * P : (j + 1) * P],
                                        identb[:],
                                    )
                                nc.scalar.activation(
                                    out=h2T8[:, :, mi * P : (mi + 1) * P],
                                    in_=tr_ps[:, :, :],
                                    func=AF.Copy,
                                )

                    # FFN1 fp8 DoubleRow; FFN2 bf16 (error budget), WS-scaled
                    gT = ph2w.tile([P, NFT, 2 * QT], BF16, name="gT")
                    if "ffn" in skip:
                        for m in range(2 * QT // P):
                            y_t = ph2.tile([P, C], F32, name="y_t")
                            nc.vector.tensor_copy(y_t, x2_sb[:, m, :])
                            nc.sync.dma_start(out=out[m * P : (m + 1) * P, :], in_=y_t)
                        return
                    with (
                        tc.tile_pool(name="ph2psB", bufs=4, space="PSUM") as ph2psB,
                        tc.tile_pool(name="ph2psY", bufs=1, space="PSUM") as ph2psY,
                    ):
                        ps_y = ph2psY.tile([P, 2 * QT // P, C], F32, name="ps_y")
                        for f in range(NFT):
                            ps_g = ph2psB.tile([P, 2 * QT], F32, name="ps_g")
                            for u in range(2):
                                nc.tensor.matmul(
                                    ps_g[:],
                                    w1_sb[:, u, :, f * P : (f + 1) * P],
                                    h2T8[:, 2 * u : 2 * u + 2, :],
                                    perf_mode=mybir.MatmulPerfMode.DoubleRow,
                                    start=(u == 0),
                                    stop=(u == 1),
                                )
                            nc.scalar.activation(
                                out=gT[:, f, :],
                                in_=ps_g[:],
                                func=AF.Relu,
                                bias=b1cols[:, f : f + 1],
                            )
                            for m in range(2 * QT // P):
                                nc.tensor.matmul(
                                    ps_y[:, m, :],
                                    gT[:, f, m * P : (m + 1) * P],
                                    w2_sb[:, f, :],
                                    start=(f == 0),
                                    stop=False,
                                )
                        for m in range(2 * QT // P):
                            # rank-1 b2 add closes the accumulation group
                            nc.tensor.matmul(
                                ps_y[:, m, :],
                                ones_row[0:1, 0:P],
                                b2_sb[0:1, :],
                                start=False,
                                stop=True,
                            )
                            y_t = ph2.tile([P, C], F32, name="y_t")
                            nc.vector.tensor_tensor(
                                out=y_t, in0=ps_y[:, m, :], in1=x2_sb[:, m, :], op=ALU.add
                            )
                            nc.vector.tensor_scalar(
                                out=y_t, in0=y_t, scalar1=1.0 / WS, scalar2=None,
                                op0=ALU.mult,
                            )
                            nc.sync.dma_start(out=out[m * P : (m + 1) * P, :], in_=y_t)


            st = emit_front()
            for _rep in range(repeat):
                a2a_out_r = emit_attn_pack(st)
                nxt = emit_front() if _rep + 1 < repeat else None
                emit_ph2(st, a2a_out_r)
                st = nxt

    _split_excess_waits(nc)
    return nc


_NC_CACHE = None


def _get_nc():
    global _NC_CACHE
    if _NC_CACHE is None:
        _NC_CACHE = _build_nc()
    return _NC_CACHE


def _make_in_maps(inputs):
    f = lambda a: np.ascontiguousarray(np.asarray(a, dtype=np.float32))
    x = f(inputs["x"])  # [B, T, C]
    Wq, Wk, Wv = f(inputs["Wq"]), f(inputs["Wk"]), f(inputs["Wv"])
    bq, bk, bv = f(inputs["bq"]), f(inputs["bk"]), f(inputs["bv"])
    Wo, bo = f(inputs["Wo"]), f(inputs["bo"])
    W1, b1 = f(inputs["W1"]), f(inputs["b1"])
    W2, b2 = f(inputs["W2"]), f(inputs["b2"])
    g1, be1 = f(inputs["g1"]), f(inputs["be1"])
    g2, be2 = f(inputs["g2"]), f(inputs["be2"])

    # LN1 fold: h = z*g1 + be1  =>  h@W = z@(g1 d W) + be1@W
    Wq_f = g1[:, None] * Wq  # [H, C, HS] broadcast over H? shape [H,C,HS]
    Wk_f = g1[:, None] * Wk
    Wv_f = g1[:, None] * Wv
    # per-head folded biases
    bq_f = np.einsum("c,hcd->hd", be1, Wq) + bq
    bk_f = np.einsum("c,hcd->hd", be1, Wk) + bk
    bv_f = np.einsum("c,hcd->hd", be1, Wv) + bv
    # LN2 fold into FFN1
    W1_f = g2[:, None] * W1
    b1_f = be2 @ W1 + b1
    # wo row permutation: payload head-slot order is [0, 2, 1, 3] (partition-
    # group major from the packed st8 layout); slot hidx of src core jsrc
    # carries true head head_at[hidx], i.e.
    # c' = 128*hidx + 32*jsrc + hs <- c = 32*(4*jsrc + head_at[hidx]) + hs
    head_at = [0, 2, 1, 3]
    perm = np.empty(C, dtype=np.int64)
    for hidx in range(HPC):
        for jsrc in range(GROUP):
            for hs in range(HS):
                perm[128 * hidx + 32 * jsrc + hs] = (
                    32 * (4 * jsrc + head_at[hidx]) + hs
                )
    Wo_p = np.ascontiguousarray(Wo[perm])

    import ml_dtypes

    def dr_layout(W):  # [R, N] -> [128, R//256, 2, N] fp8: w[k,u,j,n]=W[128*(2u+j)+k,n]
        R, N = W.shape
        w = W.reshape(R // 256, 2, P, N).transpose(2, 0, 1, 3)
        return np.ascontiguousarray(w.astype(ml_dtypes.float8_e4m3))

    shared = {
        "wo8": dr_layout(Wo_p * WS),
        "w18": dr_layout(W1_f * WS),
        "b1r": np.ascontiguousarray(b1_f).reshape(DF // P, P),
        "w2b": np.ascontiguousarray(W2.astype(ml_dtypes.bfloat16)),
        "b2": b2,
    }
    in_maps = []
    for c in range(NCORES):
        g = c // GROUP
        h0 = HPC * (c % GROUP)
        hsl = slice(h0, h0 + HPC)
        xres = (
            np.concatenate(
                [x[0, QT * c : QT * (c + 1)], x[1, QT * c : QT * (c + 1)]], axis=0
            )
            + bo[None, :]
        ) * WS
        in_maps.append(
            {
                "xfull": x[g],
                "xres": np.ascontiguousarray(xres),
                "wq": np.ascontiguousarray(
                    Wq_f[hsl].transpose(1, 0, 2).reshape(C, D2)
                ),
                "wk": np.ascontiguousarray(
                    Wk_f[hsl].transpose(1, 0, 2).reshape(C, D2)
                ),
                "wv": np.ascontiguousarray(
                    Wv_f[hsl].transpose(1, 0, 2).reshape(C, D2)
                ),
                "bq": np.ascontiguousarray(bq_f[hsl].reshape(-1)),
                "bk": np.ascontiguousarray(bk_f[hsl].reshape(-1)),
                "bv": np.ascontiguousarray(bv_f[hsl].reshape(-1)),
                **shared,
            }
        )
    return in_maps


def kernel(**inputs) -> np.ndarray:
    nc = _get_nc()
    in_maps = _make_in_maps(inputs)
    res = run_bass_kernel_spmd(nc, in_maps, list(range(NCORES)))
    out = np.empty((B, T, C), dtype=np.float32)
    for c in range(NCORES):
        r = res.results[c]["out"]
        out[0, QT * c : QT * (c + 1)] = r[0:QT]
        out[1, QT * c : QT * (c + 1)] = r[QT : 2 * QT]
    return out

